# revision 17
# baseline (speedup 1.0000x reference)
"""Cross bi-directional Mamba block (DirectionalAGLGF) on 8 Trainium2 cores.

Sharding: (batch 2) x (sequence-quarter 4). The SSM scan is sequence-parallel
with a 128-step decay warmup instead of cross-core state handoff (state decays
by >= exp(-23) over the warmup window, far below fp32 resolution).

The backward direction runs as a *forward* pipeline over host-flipped input
slices (bwd scan == causal scan on the reversed sequence). Its gated output
projection is kept separate (out = out_f + flip(out_b) is linear in the two
branches), so the flip-back happens on the host and the device never touches
a reversed access pattern (reversed APs measured ~400x slow on HW).

Per-core layout: features on partitions, sequence on the free dimension.
  - LN folded into projection weights; stats via PE ones-matmuls; rsqrt via
    exp(-0.5*ln(v)) (single ACT table set: {exp, ln, square, copy, identity});
    row-to-tile broadcasts via K=1 matmuls (gpsimd partition_broadcast is
    ~2.5 ms/call here).
  - causal conv folded into the input projection (4 shifted accumulating
    matmuls with conv-premultiplied weights).
  - silu(x) = x * exp(-ln(1+exp(-x))), softplus(x) = ln(1+exp(x)).
  - scan state tiles pack 32 d-channels x 4 n-channels per 128 partitions;
    dt/dtu expanded across n by 0/1 matmuls (fp32r), B/C expanded across d
    by replicating DMA reads on the sync queue.
  - recurrence via the DVE tensor_tensor_scan instruction (in-place over the
    dBu tile).
  - y = sum_n C*h + u*D via block-ones / D-scaled-selection matmuls
    accumulated in PSUM.
"""
import sys
sys.path.insert(0, '/opt/trn_rl_repo')
sys.path.insert(0, '/root/.axon_site/_ro/trn_rl_repo')
import numpy as np

B, C, HW, L = 2, 128, 64, 4096
D, N, R, K = 256, 16, 8, 4
Lo, W = 1024, 128
SW = Lo + W            # scan window 1152
XW = Lo + 2 * W + 16   # x window 1296
CHUNKS = [(0, 512), (512, 512), (1024, SW - 1024)]
XCH = [(0, 512), (512, 512), (1024, XW - 1024)]
OCH = [(0, 512), (512, 512)]
TAPS = [5, 6, 7, 8]    # conv tap offsets (causal, both dirs after flip)
OO = W                 # owned slice start within scan window

_STATE = {}
VARIANT = ''


def _prep_params(p):
    """Host-side parameter folding (numpy, tiny)."""
    f32 = np.float32
    out = {}
    ln_q_w, ln_q_b = p['ln_q_w'], p['ln_q_b']
    ln_kv_w, ln_kv_b = p['ln_kv_w'], p['ln_kv_b']
    w_in_x, w_in_z = p['w_in_x'], p['w_in_z']
    conv_w = [p['conv_w'], p['conv_w_b']]
    conv_b = [p['conv_b'], p['conv_b_b']]
    xpw = [p['x_proj_w'], p['x_proj_w_b']]
    dtw = [p['dt_w'], p['dt_w_b']]
    dtb = [p['dt_b'], p['dt_b_b']]
    A_log = [p['A_log'], p['A_log_b']]
    Dp = [p['D'], p['D_b']]

    wx_ln = w_in_x * ln_q_w[None, :]          # (256,128)
    t_x = w_in_x @ ln_q_b                     # (256,)
    wG = np.zeros((2, K, 128, D), f32)        # lhsT (c, d) per dir,k
    bias_x = np.zeros((2, 2, 128, 1), f32)    # (dir, dchunk, 128, 1)
    for dr in range(2):
        for k in range(K):
            wG[dr, k] = (conv_w[dr][:, k:k + 1] * wx_ln).T
        bx = conv_b[dr] + t_x * conv_w[dr].sum(axis=1)
        bias_x[dr] = bx.reshape(2, 128, 1)
    out['wG'] = wG
    out['bias_x'] = bias_x
    out['neg_bias_x'] = -bias_x
    out['wZ'] = (w_in_z * ln_kv_w[None, :]).T.astype(f32).copy()   # (128,256)
    bz = (w_in_z @ ln_kv_b).astype(f32)
    out['bias_z'] = bz.reshape(2, 128, 1)
    out['neg_bias_z'] = -bz.reshape(2, 128, 1)
    out['xpwT'] = np.stack([w.T for w in xpw]).astype(f32)         # (2,256,40)
    out['dtwT'] = np.stack([w.T for w in dtw]).astype(f32)         # (2,8,256)
    out['dtb'] = np.stack(dtb).astype(f32).reshape(2, 2, 128, 1)
    A = [-np.exp(a).astype(f32) for a in A_log]                    # (256,16)
    acols = np.zeros((2, 128, 32), f32)
    pidx = np.arange(128)
    for dr in range(2):
        for t in range(32):
            g, nq = t // 4, t % 4
            acols[dr, :, t] = A[dr][32 * g + pidx % 32, 4 * nq + pidx // 32]
    out['A_cols'] = acols
    eq = np.zeros((128, 512), f32)
    for gq in range(4):
        for pp in range(128):
            eq[32 * gq + pp % 32, 128 * gq + pp] = 1.0
    out['Eq'] = eq
    ones_red = np.zeros((128, 32), f32)
    ones_red[pidx, pidx % 32] = 1.0
    out['ones_red'] = ones_red
    # D-scaled selection lhsT folding u*D into the PSUM reduction
    dsel = np.zeros((2, 8, 128, 32), f32)
    for dr in range(2):
        for g in range(8):
            for m in range(32):
                dsel[dr, g, 32 * (g % 4) + m, m] = Dp[dr][32 * g + m]
    out['D_sel'] = dsel
    out['outwT'] = p['out_w'].T.astype(f32).copy()                 # (256,128)
    out['out_b'] = p['out_b'].astype(f32).reshape(128, 1)
    return out


def _build(nc, reps=1):
    import concourse.mybir as mybir
    import concourse.tile as tile
    f32 = mybir.dt.float32
    f32r = mybir.dt.float32r
    Alu = mybir.AluOpType
    AF = mybir.ActivationFunctionType
    Exp, Ln, Sq, Ident = AF.Exp, AF.Ln, AF.Square, AF.Identity

    dp = nc.declare_dram_parameter
    d_x = [dp("x1s", [128, XW], f32, isOutput=False),
           dp("x1sr", [128, XW], f32, isOutput=False)]
    d_x2 = [dp("x2s", [128, XW], f32, isOutput=False),
            dp("x2sr", [128, XW], f32, isOutput=False)]
    d_mask = [dp("mask0", [128, 512], f32, isOutput=False),
              dp("mask0r", [128, 512], f32, isOutput=False)]
    d_wG = dp("wG", [2, K, 128, D], f32, isOutput=False)
    d_bx = dp("bias_x", [2, 2, 128, 1], f32, isOutput=False)
    d_nbx = dp("neg_bias_x", [2, 2, 128, 1], f32, isOutput=False)
    d_wZ = dp("wZ", [128, D], f32, isOutput=False)
    d_bz = dp("bias_z", [2, 128, 1], f32, isOutput=False)
    d_nbz = dp("neg_bias_z", [2, 128, 1], f32, isOutput=False)
    d_xpwT = dp("xpwT", [2, D, 40], f32, isOutput=False)
    d_dtwT = dp("dtwT", [2, R, D], f32, isOutput=False)
    d_dtb = dp("dtb", [2, 2, 128, 1], f32, isOutput=False)
    d_ac = dp("A_cols", [2, 128, 32], f32, isOutput=False)
    d_eq = dp("Eq", [128, 512], f32, isOutput=False)
    d_or = dp("ones_red", [128, 32], f32, isOutput=False)
    d_dsel = dp("D_sel", [2, 8, 128, 32], f32, isOutput=False)
    d_ow = dp("outwT", [D, 128], f32, isOutput=False)
    d_ob = dp("out_b", [128, 1], f32, isOutput=False)
    d_outf = dp("out_f", [128, Lo], f32, isOutput=True)
    d_outb = dp("out_b_flip", [128, Lo], f32, isOutput=True)

    with tile.TileContext(nc) as tc:
        with (tc.tile_pool(name="cp", bufs=1) as cp,
              tc.tile_pool(name="mp", bufs=1) as mp,
              tc.tile_pool(name="ps", bufs=1, space="PSUM") as ps):

            def t5(name):
                return mp.tile([128, 512], f32, name=name, tag="tmp5", bufs=3)

            # ---------------- weights / consts ----------------
            def load_r(name, shape, src_ap):
                stg = mp.tile([128, 512], f32, name=f"stg_{name}", tag="tmp5", bufs=3)
                nc.sync.dma_start(stg[:shape[0], :shape[1]], src_ap)
                t = cp.tile(list(shape), f32r, name=name)
                nc.vector.tensor_copy(t[:], stg[:shape[0], :shape[1]])
                return t

            wG_t = [[[load_r(f"wG{dr}{k}{dc}", (128, 128),
                             d_wG[dr, k, :, 128 * dc:128 * dc + 128])
                      for dc in range(2)] for k in range(K)] for dr in range(2)]
            wZ_t = [load_r(f"wZ{dc}", (128, 128), d_wZ[:, 128 * dc:128 * dc + 128])
                    for dc in range(2)]
            xpwT_t = [[load_r(f"xpw{dr}{dc}", (128, 40),
                              d_xpwT[dr, 128 * dc:128 * dc + 128, :])
                       for dc in range(2)] for dr in range(2)]
            dtwT_t = [[load_r(f"dtw{dr}{dc}", (R, 128),
                              d_dtwT[dr, :, 128 * dc:128 * dc + 128])
                       for dc in range(2)] for dr in range(2)]
            eq_t = load_r("eqt", (128, 512), d_eq[:, :])
            or_t = load_r("ort", (128, 32), d_or[:, :])
            dsel_t = [[load_r(f"dsel{dr}{g}", (128, 32), d_dsel[dr, g, :, :])
                       for g in range(8)] for dr in range(2)]
            ow_t = [load_r(f"ow{dc}", (128, 128), d_ow[128 * dc:128 * dc + 128, :])
                    for dc in range(2)]

            def load_f(name, shape, src_ap):
                t = cp.tile(list(shape), f32, name=name)
                nc.sync.dma_start(t[:], src_ap)
                return t

            bz_t = [load_f(f"bzt{dc}", (128, 1), d_bz[dc, :, :]) for dc in range(2)]
            nbz_t = [load_f(f"nbzt{dc}", (128, 1), d_nbz[dc, :, :]) for dc in range(2)]
            dtb_t = [[load_f(f"dtbt{dr}{dc}", (128, 1), d_dtb[dr, dc, :, :])
                      for dc in range(2)] for dr in range(2)]
            bx_t = [[load_f(f"bxt{dr}{dc}", (128, 1), d_bx[dr, dc, :, :])
                     for dc in range(2)] for dr in range(2)]
            nbx_t = [[load_f(f"nbxt{dr}{dc}", (128, 1), d_nbx[dr, dc, :, :])
                      for dc in range(2)] for dr in range(2)]
            ac_t = [load_f(f"act{dr}", (128, 32), d_ac[dr, :, :]) for dr in range(2)]
            ob_t = load_f("obt", (128, 1), d_ob[:, :])
            mk_t = [load_f(f"mkt{dr}", (128, 512), d_mask[dr][:, :]) for dr in range(2)]
            ones1 = cp.tile([128, 1], f32, name="ones1")
            nc.vector.memset(ones1[:], 1.0)
            onesr = cp.tile([1, 128], f32, name="onesr")
            nc.vector.memset(onesr[:], 1.0)
            eps_t = cp.tile([128, 1], f32, name="eps_t")
            nc.vector.memset(eps_t[:], 1e-5)

            # ---------------- body ----------------
            def rowc(name):
                return mp.tile([1, 512], f32, name=name, tag="rowc", bufs=5)

            def layernorm(d_in, out_name):
                """x -> (x - mu) * rsqrt(var+eps), f32r, (128, XW)."""
                raw = mp.tile([128, XW], f32, name=f"raw_{out_name}", tag="w1296", bufs=2)
                nc.sync.dma_start(raw[:], d_in[:, :])
                xn = mp.tile([128, XW], f32r, name=out_name, tag="xn", bufs=3)
                for (s, ln) in XCH:
                    sq = t5(f"sq_{out_name}{s}")
                    nc.scalar.activation(sq[:, :ln], raw[:, s:s + ln], Sq)
                    p1 = ps.tile([1, 512], f32, name=f"pst1_{out_name}{s}", tag="red", bufs=2)
                    p2 = ps.tile([1, 512], f32, name=f"pst2_{out_name}{s}", tag="red", bufs=2)
                    nc.tensor.matmul(p1[:, :ln], ones1[:], raw[:, s:s + ln],
                                     start=True, stop=True)
                    nc.tensor.matmul(p2[:, :ln], ones1[:], sq[:, :ln],
                                     start=True, stop=True)
                    mu = rowc(f"mu_{out_name}{s}")
                    msq = rowc(f"msq_{out_name}{s}")
                    nc.scalar.mul(mu[:, :ln], p1[:, :ln], 1.0 / 128)
                    nc.scalar.mul(msq[:, :ln], p2[:, :ln], 1.0 / 128)
                    mu2 = rowc(f"mu2_{out_name}{s}")
                    nc.scalar.activation(mu2[:, :ln], mu[:, :ln], Sq)
                    var = rowc(f"var_{out_name}{s}")
                    nc.vector.tensor_tensor(var[:, :ln], msq[:, :ln], mu2[:, :ln],
                                            Alu.subtract)
                    lnv = rowc(f"lnv_{out_name}{s}")
                    nc.scalar.activation(lnv[:, :ln], var[:, :ln], Ln, bias=eps_t[:1, :])
                    r = rowc(f"r_{out_name}{s}")
                    nc.scalar.activation(r[:, :ln], lnv[:, :ln], Exp, scale=-0.5)
                    mur = rowc(f"mur_{out_name}{s}")
                    nc.vector.tensor_tensor(mur[:, :ln], mu[:, :ln], r[:, :ln],
                                            Alu.mult)
                    # broadcast rows to 128 partitions via K=1 matmuls
                    rb = ps.tile([128, 512], f32, name=f"rb_{out_name}{s}",
                                 tag="exp", bufs=4)
                    murb = ps.tile([128, 512], f32, name=f"murb_{out_name}{s}",
                                   tag="exp", bufs=4)
                    nc.tensor.matmul(rb[:, :ln], onesr[:], r[:, :ln],
                                     start=True, stop=True)
                    nc.tensor.matmul(murb[:, :ln], onesr[:], mur[:, :ln],
                                     start=True, stop=True)
                    tmp = t5(f"tmpn_{out_name}{s}")
                    nc.vector.tensor_tensor(tmp[:, :ln], raw[:, s:s + ln],
                                            rb[:, :ln], Alu.mult)
                    nc.vector.tensor_tensor(xn[:, s:s + ln], tmp[:, :ln],
                                            murb[:, :ln], Alu.subtract)
                return xn

            def z_branch(x2n, dr):
                """silu(z) on the owned range, from normalized x2."""
                zst = mp.tile([128, 2 * Lo], f32, name=f"zs{dr}", tag="zs", bufs=2)
                zs = [zst[:, :Lo], zst[:, Lo:]]
                for dc in range(2):
                    for (s, ln) in OCH:
                        pz = ps.tile([128, 512], f32, name=f"pz{dr}{dc}{s}",
                                     tag="mm", bufs=2)
                        nc.tensor.matmul(pz[:, :ln], wZ_t[dc][:],
                                         x2n[:, 136 + s:136 + s + ln],
                                         start=True, stop=True)
                        e = t5(f"ze{dr}{dc}{s}")
                        nc.scalar.activation(e[:, :ln], pz[:, :ln], Exp, scale=-1.0,
                                             bias=nbz_t[dc][:])
                        sp = t5(f"zsp{dr}{dc}{s}")
                        nc.scalar.activation(sp[:, :ln], e[:, :ln], Ln, bias=1.0)
                        sg = t5(f"zsg{dr}{dc}{s}")
                        nc.scalar.activation(sg[:, :ln], sp[:, :ln], Exp, scale=-1.0)
                        nc.vector.scalar_tensor_tensor(
                            zs[dc][:, s:s + ln], pz[:, :ln], bz_t[dc][:],
                            sg[:, :ln], Alu.add, Alu.mult)
                return zs

            def direction(dr, x1n, zs, d_outx):
                """Full causal pipeline for one direction -> gated projected
                output DMA'd to d_outx."""
                xc = [mp.tile([128, SW], f32r, name=f"xc{dr}{dc}", tag="xc", bufs=3)
                      for dc in range(2)]
                for dc in range(2):
                    for ci, (s, ln) in enumerate(CHUNKS):
                        px = ps.tile([128, 512], f32, name=f"px{dr}{dc}{s}",
                                     tag="mm", bufs=2)
                        for k in range(K):
                            t0 = TAPS[k] + s
                            nc.tensor.matmul(px[:, :ln], wG_t[dr][k][dc][:],
                                             x1n[:, t0:t0 + ln],
                                             start=(k == 0), stop=(k == K - 1))
                        e = t5(f"xe{dr}{dc}{s}")
                        nc.scalar.activation(e[:, :ln], px[:, :ln], Exp, scale=-1.0,
                                             bias=nbx_t[dr][dc][:])
                        sp = t5(f"xsp{dr}{dc}{s}")
                        nc.scalar.activation(sp[:, :ln], e[:, :ln], Ln, bias=1.0)
                        sg = t5(f"xsg{dr}{dc}{s}")
                        nc.scalar.activation(sg[:, :ln], sp[:, :ln], Exp, scale=-1.0)
                        nc.vector.scalar_tensor_tensor(
                            xc[dc][:, s:s + ln], px[:, :ln], bx_t[dr][dc][:],
                            sg[:, :ln], Alu.add, Alu.mult)

                # x_proj -> dbl (dt_r 8 | B 16 | C 16)
                dbl = mp.tile([40, SW], f32r, name=f"dbl{dr}", tag="dbl", bufs=1)
                for ci, (s, ln) in enumerate(CHUNKS):
                    p40 = ps.tile([40, 512], f32, name=f"p40_{dr}{s}", tag="mm", bufs=2)
                    for dc in range(2):
                        nc.tensor.matmul(p40[:, :ln], xpwT_t[dr][dc][:],
                                         xc[dc][:, s:s + ln],
                                         start=(dc == 0), stop=(dc == 1))
                    nc.scalar.copy(dbl[:, s:s + ln], p40[:, :ln])

                # B_exp / C_exp by replicating DMA (sync queue)
                bexp, cexp = [], []
                for nq in range(4):
                    bx = mp.tile([128, SW], f32, name=f"bex{dr}{nq}", tag="bex", bufs=4)
                    cx = mp.tile([128, Lo], f32, name=f"cex{dr}{nq}", tag="cex", bufs=4)
                    if VARIANT == "nodma":
                        nc.vector.memset(bx[:], 0.01)
                        nc.vector.memset(cx[:], 0.01)
                    else:
                        src = dbl[8 + 4 * nq:12 + 4 * nq, :].bitcast(f32)
                        nc.sync.dma_start(bx[:], src.unsqueeze(1).to_broadcast((4, 32, SW)))
                        csrc = dbl[24 + 4 * nq:28 + 4 * nq, OO:OO + Lo].bitcast(f32)
                        nc.sync.dma_start(cx[:], csrc.unsqueeze(1).to_broadcast((4, 32, Lo)))
                    bexp.append(bx)
                    cexp.append(cx)

                # per d-chunk: dt/dtu chunks, then its 4 groups
                ydir = [mp.tile([128, Lo], f32, name=f"yd{dr}{dc}", tag="ydir", bufs=2)
                        for dc in range(2)]
                for dc in range(2):
                    dtt, dtu = [], []
                    for ci, (s, ln) in enumerate(CHUNKS):
                        pd = ps.tile([128, 512], f32, name=f"pd{dr}{dc}{s}",
                                     tag="mm", bufs=2)
                        nc.tensor.matmul(pd[:, :ln], dtwT_t[dr][dc][:],
                                         dbl[0:8, s:s + ln], start=True, stop=True)
                        e = t5(f"de{dr}{dc}{s}")
                        nc.scalar.activation(e[:, :ln], pd[:, :ln], Exp,
                                             bias=dtb_t[dr][dc][:])
                        dt_c = mp.tile([128, 512], f32r, name=f"dt{dr}{dc}{s}",
                                       tag="dtc", bufs=4)
                        if ci == 0:
                            spt = t5(f"dsp{dr}{dc}{s}")
                            nc.scalar.activation(spt[:, :ln], e[:, :ln], Ln, bias=1.0)
                            nc.vector.tensor_tensor(dt_c[:, :ln], spt[:, :ln],
                                                    mk_t[dr][:, :ln], Alu.mult)
                        else:
                            nc.scalar.activation(dt_c[:, :ln], e[:, :ln], Ln, bias=1.0)
                        du_c = mp.tile([128, 512], f32r, name=f"du{dr}{dc}{s}",
                                       tag="duc", bufs=4)
                        nc.vector.tensor_tensor(du_c[:, :ln], dt_c[:, :ln],
                                                xc[dc][:, s:s + ln], Alu.mult)
                        dtt.append(dt_c)
                        dtu.append(du_c)

                    for gq in range(4):
                        g = 4 * dc + gq
                        pe_dt = []
                        due_s = mp.tile([128, SW], f32, name=f"due{dr}{g}",
                                        tag="due", bufs=2)
                        for ci, (s, ln) in enumerate(CHUNKS):
                            pdt = ps.tile([128, 512], f32, name=f"pdt{dr}{g}{s}",
                                          tag="exp", bufs=4)
                            nc.tensor.matmul(pdt[:, :ln],
                                             eq_t[:, 128 * gq:128 * gq + 128],
                                             dtt[ci][:, :ln], start=True, stop=True)
                            pe_dt.append(pdt)
                            pdu = ps.tile([128, 512], f32, name=f"pdu{dr}{g}{s}",
                                          tag="exp", bufs=4)
                            nc.tensor.matmul(pdu[:, :ln],
                                             eq_t[:, 128 * gq:128 * gq + 128],
                                             dtu[ci][:, :ln], start=True, stop=True)
                            nc.scalar.copy(due_s[:, s:s + ln], pdu[:, :ln])
                        red = [ps.tile([32, 512], f32, name=f"red{dr}{g}{lc}",
                                       tag="red", bufs=2) for lc in range(2)]
                        for nq in range(4):
                            t = g * 4 + nq
                            dA = mp.tile([128, SW], f32, name=f"dA{dr}{t}",
                                         tag="dA", bufs=2)
                            for ci, (s, ln) in enumerate(CHUNKS):
                                nc.scalar.activation(dA[:, s:s + ln], pe_dt[ci][:, :ln],
                                                     Exp, scale=ac_t[dr][:, t:t + 1])
                            dB = mp.tile([128, SW], f32, name=f"dB{dr}{t}",
                                         tag="dB", bufs=2)
                            nc.vector.tensor_tensor(dB[:], due_s[:], bexp[nq][:],
                                                    Alu.mult)
                            # scan in-place over dB (forward only)
                            if VARIANT == "noscan":
                                nc.vector.tensor_tensor(dB[:], dA[:], dB[:], Alu.mult)
                            else:
                                nc.vector.tensor_tensor_scan(dB[:], dA[:], dB[:], 0.0,
                                                             Alu.mult, Alu.add)
                            pr = mp.tile([128, Lo], f32r, name=f"pr{dr}{t}",
                                         tag="pr", bufs=2)
                            nc.vector.tensor_tensor(pr[:], dB[:, OO:OO + Lo],
                                                    cexp[nq][:], Alu.mult)
                            for lc in range(2):
                                nc.tensor.matmul(red[lc][:, :], or_t[:],
                                                 pr[:, 512 * lc:512 * lc + 512],
                                                 start=(nq == 0), stop=False)
                        # fold u*D via D-scaled selection matmul (closes group)
                        for lc in range(2):
                            nc.tensor.matmul(red[lc][:, :], dsel_t[dr][g][:],
                                             xc[dc][:, OO + 512 * lc:OO + 512 * lc + 512],
                                             start=False, stop=True)
                            nc.scalar.copy(
                                ydir[dc][32 * gq:32 * gq + 32, 512 * lc:512 * lc + 512],
                                red[lc][:, :])

                # gate with silu(z) and project
                outs = mp.tile([128, Lo], f32, name=f"outs{dr}", tag="outs", bufs=2)
                yg = []
                for dc in range(2):
                    ygt = mp.tile([128, Lo], f32r, name=f"yg{dr}{dc}", tag="yg", bufs=2)
                    nc.vector.tensor_tensor(ygt[:], ydir[dc][:], zs[dc][:], Alu.mult)
                    yg.append(ygt)
                for (s, ln) in OCH:
                    po = ps.tile([128, 512], f32, name=f"po{dr}{s}", tag="mm", bufs=2)
                    for dc in range(2):
                        nc.tensor.matmul(po[:, :ln], ow_t[dc][:], yg[dc][:, s:s + ln],
                                         start=(dc == 0), stop=(dc == 1))
                    if dr == 0:
                        nc.scalar.activation(outs[:, s:s + ln], po[:, :ln], Ident,
                                             bias=ob_t[:])
                    else:
                        nc.scalar.copy(outs[:, s:s + ln], po[:, :ln])
                nc.sync.dma_start(d_outx[:, :], outs[:])

            def emit_body():
                x2n = layernorm(d_x2[0], "x2n")
                zs_f = z_branch(x2n, 0)
                x1n = layernorm(d_x[0], "x1n")
                direction(0, x1n, zs_f, d_outf)
                x2nr = layernorm(d_x2[1], "x2nr")
                zs_b = z_branch(x2nr, 1)
                x1nr = layernorm(d_x[1], "x1nr")
                direction(1, x1nr, zs_b, d_outb)

            for _ in range(reps):
                emit_body()
    return nc


def _make_in_maps(x1, x2, params):
    x1f = np.ascontiguousarray(x1.reshape(B, 128, L)).astype(np.float32)
    x2f = np.ascontiguousarray(x2.reshape(B, 128, L)).astype(np.float32)
    x1r = x1f[:, :, ::-1]
    x2r = x2f[:, :, ::-1]

    def slice_q(arr, b, q):
        lo = 1024 * q - (W + 8)
        sl = np.zeros((128, XW), np.float32)
        a, bnd = max(0, lo), min(L, lo + XW)
        sl[:, a - lo:bnd - lo] = arr[b][:, a:bnd]
        return sl, lo

    in_maps = []
    for core in range(8):
        b, q = core // 4, core % 4
        s1, lo = slice_q(x1f, b, q)
        s2, _ = slice_q(x2f, b, q)
        qr = 3 - q
        s1r, lor = slice_q(x1r, b, qr)
        s2r, _ = slice_q(x2r, b, qr)
        # mask over scan-window j in [0,512): valid iff 0 <= lo+8+j < L
        jj = lo + 8 + np.arange(512)
        m0 = np.broadcast_to(((jj >= 0) & (jj < L)).astype(np.float32),
                             (128, 512)).copy()
        jjr = lor + 8 + np.arange(512)
        m0r = np.broadcast_to(((jjr >= 0) & (jjr < L)).astype(np.float32),
                              (128, 512)).copy()
        m = {"x1s": s1, "x2s": s2, "x1sr": s1r, "x2sr": s2r,
             "mask0": m0, "mask0r": m0r}
        m.update(params)
        in_maps.append(m)
    return in_maps


def _assemble(res, x2):
    out = np.zeros((B, 128, L), np.float32)
    for core in range(8):
        b, q = core // 4, core % 4
        of = res[core]["out_f"]
        ob = res[core]["out_b_flip"][:, ::-1]
        out[b][:, 1024 * q:1024 * (q + 1)] = of + ob
    return out.reshape(B, 128, HW, HW), x2


def kernel(**inputs):
    import concourse.bacc as bacc
    from concourse.bass_utils import run_bass_kernel_spmd

    x1, x2 = inputs['x1'], inputs['x2']
    params = _prep_params(inputs)

    if 'nc' not in _STATE:
        nc = bacc.Bacc("TRN2", target_bir_lowering=False, debug=False)
        _build(nc)
        nc.compile()
        _STATE['nc'] = nc
    nc = _STATE['nc']

    in_maps = _make_in_maps(x1, x2, params)
    res = run_bass_kernel_spmd(nc, in_maps, list(range(8))).results
    return _assemble(res, x2)
